# revision 15
# baseline (speedup 1.0000x reference)
"""Multi-head attention block (QKV proj + masked softmax + out proj + residual LN)
on 8 Trainium2 NeuronCores.

Sharding: 8 shards = (batch b, query-half); B=4, S=2048. Each core owns one
batch's full K/V and half its queries; no collectives, host concatenates.

Key compaction: masked keys contribute exactly 0 to the softmax numerator and
denominator, and key order inside the sums is irrelevant — so the host gathers
only the unmasked keys (<=1046 of 2048 here) and pads to S_KV=1280. Pad slots
get a -30000 exp bias -> exp underflows to exactly 0. Cuts score/exp/PV work
to 10/16 of full, mathematically exact.

Per-core strategy (all matmuls bf16 inputs, fp32 PSUM accumulation):
  - xT staged on host; projections contract d on partitions.
  - kT/qT per head-PAIR [128, S] (head h -> partitions (h%2)*64..);
    temperature and the k-bias are folded into the kT store.
  - scores transposed [k, q]: pad mask folded into exp bias, row sums via a
    ones-column in V. PSUM tiles 2 banks wide; one exp covers [128, 1024]
    (both q-tiles of a k-tile share the bias), halving ACT overhead.
  - PV contracts k on partitions; normalization = reciprocal_approx_fast of
    the sums row + gpsimd partition-broadcast + DVE multiply.
  - y = attn_out @ wo.T via K=128 head-pair contractions, then residual
    (+bo folded into x host-side, kept fp32) and LayerNorm (bn_stats/aggr).
  - Build-time specialization on the actual inputs: gamma==1/beta==0 and
    bv==0 drop their (otherwise dead) ops.
"""

import os
import numpy as np
import ml_dtypes

import concourse.bass as bass
import concourse.bacc as bacc
import concourse.tile as tile
import concourse.mybir as mybir
from concourse.bass_utils import run_bass_kernel_spmd

F32 = mybir.dt.float32
BF16 = mybir.dt.bfloat16
AF = mybir.ActivationFunctionType
ALU = mybir.AluOpType

B, S, D = 4, 2048, 512
H, HD = 8, 64
NCORES = 8
SQ = S // 2          # queries per core
NP = 4               # head pairs
NQT = SQ // 512      # 2 q-tiles of 512
NST = SQ // 128      # 8 output s-tiles

_CACHE = {}
LAST_RESULT = None


def _build(ln_trivial, bv_trivial, S_KV):
    NKT = S_KV // 128
    key = ("nc", ln_trivial, bv_trivial, S_KV)
    if key in _CACHE:
        return _CACHE[key]

    nc = bacc.Bacc("TRN2", target_bir_lowering=False, debug=False, num_devices=NCORES)

    xTk = nc.dram_tensor("xTk", [D, S_KV], BF16, kind="ExternalInput")
    xTq = nc.dram_tensor("xTq", [D, SQ], BF16, kind="ExternalInput")
    xq = nc.dram_tensor("xq", [SQ, D], F32, kind="ExternalInput")
    wqT = nc.dram_tensor("wqT", [D, D], BF16, kind="ExternalInput")
    wkT = nc.dram_tensor("wkT", [D, D], BF16, kind="ExternalInput")
    wvT = nc.dram_tensor("wvT", [D, D], BF16, kind="ExternalInput")
    woT = nc.dram_tensor("woT", [D, D], BF16, kind="ExternalInput")
    bqk = nc.dram_tensor("bqk", [128, 8], F32, kind="ExternalInput")
    if not bv_trivial:
        bv_row = nc.dram_tensor("bv_row", [1, D], BF16, kind="ExternalInput")
    maskb = nc.dram_tensor("maskb", [128, NKT], F32, kind="ExternalInput")
    temp_b = nc.dram_tensor("temp_b", [128, 1], F32, kind="ExternalInput")
    if not ln_trivial:
        gamma = nc.dram_tensor("gamma", [1, D], F32, kind="ExternalInput")
        beta = nc.dram_tensor("beta", [1, D], F32, kind="ExternalInput")
    out = nc.dram_tensor("out", [SQ, D], F32, kind="ExternalOutput")

    def dram_bcast(t, p=128):
        a = t.ap()
        return bass.AP(tensor=a.tensor, offset=a.offset, ap=[[0, p]] + list(a.ap)[1:])

    with tile.TileContext(nc) as tc, nc.allow_low_precision(reason="bf16 matmuls"):
        with tc.tile_pool(name="consts", bufs=1) as consts, \
             tc.tile_pool(name="kqv", bufs=1) as kqv, \
             tc.tile_pool(name="proj", bufs=1) as proj, \
             tc.tile_pool(name="attn", bufs=3) as attn, \
             tc.tile_pool(name="psmm", bufs=3, space="PSUM") as psmm, \
             tc.tile_pool(name="pspv", bufs=2, space="PSUM") as pspv, \
             tc.tile_pool(name="small", bufs=2) as small:

            # ---- constants (small, fast DMAs first) ----
            bqk_t = consts.tile([128, 8], F32, tag="bqk")
            nc.sync.dma_start(out=bqk_t, in_=bqk[:, :])
            mb_t = consts.tile([128, NKT], F32, tag="mb")
            nc.sync.dma_start(out=mb_t, in_=maskb[:, :])
            tp_t = consts.tile([128, 1], F32, tag="tp")
            nc.sync.dma_start(out=tp_t, in_=temp_b[:, :])
            if not bv_trivial:
                bv_t = consts.tile([1, D], BF16, tag="bv")
                nc.sync.dma_start(out=bv_t, in_=bv_row[:, :])
            if not ln_trivial:
                g_t = consts.tile([128, D], F32, tag="g")
                nc.sync.dma_start(out=g_t, in_=dram_bcast(gamma))
                b_t = consts.tile([128, D], F32, tag="b")
                nc.sync.dma_start(out=b_t, in_=dram_bcast(beta))
            eps_t = consts.tile([128, 1], F32, tag="eps")
            nc.vector.memset(eps_t, 1e-6)
            ones_f = consts.tile([128, 128], F32, tag="onesf")
            nc.vector.memset(ones_f, 1.0)
            ones_b = consts.tile([1, 128], BF16, tag="onesb")
            nc.vector.tensor_copy(out=ones_b, in_=ones_f[0:1, :])

            # ---- persistent activations ----
            kT = [kqv.tile([128, S_KV], BF16, tag=f"kT{p}", name=f"kT{p}")
                  for p in range(NP)]
            qT = [kqv.tile([128, SQ], BF16, tag=f"qT{p}", name=f"qT{p}")
                  for p in range(NP)]
            v_all = kqv.tile([128, H, NKT, HD + 1], BF16, tag="vall")
            outn = kqv.tile([128, NP, SQ], BF16, tag="outn")

            # ---- input staging: per-chunk DMAs so compute starts early ----
            wv_t = proj.tile([128, 4, D], BF16, tag="wv")
            xtk = proj.tile([128, 4, S_KV], BF16, tag="xtk")
            wk_t = proj.tile([128, 4, D], BF16, tag="wk")
            wq_t = proj.tile([128, 4, D], BF16, tag="wq")
            xtq = proj.tile([128, 4, SQ], BF16, tag="xtq")
            wo_t = consts.tile([128, 4, D], BF16, tag="wo")
            for c in range(4):
                nc.sync.dma_start(out=wv_t[:, c, :], in_=wvT[c * 128:(c + 1) * 128, :])
            for c in range(4):
                nc.gpsimd.dma_start(out=xtk[:, c, :], in_=xTk[c * 128:(c + 1) * 128, :])
            for c in range(4):
                nc.sync.dma_start(out=wk_t[:, c, :], in_=wkT[c * 128:(c + 1) * 128, :])
            for c in range(4):
                nc.sync.dma_start(out=wq_t[:, c, :], in_=wqT[c * 128:(c + 1) * 128, :])
            for c in range(4):
                nc.gpsimd.dma_start(out=xtq[:, c, :], in_=xTq[c * 128:(c + 1) * 128, :])
            for c in range(4):
                nc.sync.dma_start(out=wo_t[:, c, :], in_=woT[c * 128:(c + 1) * 128, :])

            # ---- V projection: all heads at once (+ ones-row bias matmul) ----
            nc.vector.tensor_copy(out=v_all[:, :, :, HD:HD + 1],
                                  in_=ones_f[:, 0:H * NKT])
            for t2 in range((NKT + 1) // 2):
                ts = [t for t in (2 * t2, 2 * t2 + 1) if t < NKT]
                ps = psmm.tile([128, 2, 512], F32, tag="mm")
                for j, t in enumerate(ts):
                    for c in range(4):
                        nc.tensor.matmul(
                            ps[:, j, :], xtk[:, c, t * 128:(t + 1) * 128],
                            wv_t[:, c, :], start=(c == 0),
                            stop=(c == 3 and bv_trivial))
                    if not bv_trivial:
                        nc.tensor.matmul(ps[:, j, :], ones_b[0:1, :], bv_t,
                                         start=False, stop=True)
                for h in range(H):
                    nc.vector.tensor_copy(
                        out=v_all[:, h, ts[0]:ts[0] + len(ts), 0:HD],
                        in_=ps[:, 0:len(ts), h * HD:(h + 1) * HD])

            def emit_kq(p):
                # kT store folds +bk and *temperature (exact when temp=2^-k)
                kv_groups = []
                off0 = 0
                while off0 < S_KV:
                    if S_KV - off0 >= 1024:
                        kv_groups.append((off0, (512, 512))); off0 += 1024
                    else:
                        kv_groups.append((off0, (S_KV - off0,))); off0 += S_KV - off0
                for g0, widths in kv_groups:
                    ps = psmm.tile([128, 2, 512], F32, tag="mm", name=f"psk{p}{g0}")
                    off = g0
                    for j, w in enumerate(widths):
                        for c in range(4):
                            nc.tensor.matmul(
                                ps[:, j, 0:w], wk_t[:, c, p * 128:(p + 1) * 128],
                                xtk[:, c, off:off + w],
                                start=(c == 0), stop=(c == 3))
                        off += w
                    tot = sum(widths)
                    src = ps if len(widths) == 2 else ps[:, 0, 0:tot]
                    nc.vector.tensor_scalar(
                        out=kT[p][:, g0:g0 + tot], in0=src,
                        scalar1=bqk_t[:, 4 + p:5 + p], scalar2=tp_t[:, 0:1],
                        op0=ALU.add, op1=ALU.mult)
                ps = psmm.tile([128, 2, 512], F32, tag="mm", name=f"psq{p}")
                for j in range(2):
                    for c in range(4):
                        nc.tensor.matmul(
                            ps[:, j, :], wq_t[:, c, p * 128:(p + 1) * 128],
                            xtq[:, c, j * 512:(j + 1) * 512],
                            start=(c == 0), stop=(c == 3))
                nc.vector.tensor_scalar_add(
                    out=qT[p][:, :], in0=ps, scalar1=bqk_t[:, p:p + 1])

            def emit_pv_chunk(prev, c):
                ph, pse, ppvs = prev
                for qt in range(NQT):
                    nc.tensor.matmul(
                        ppvs[qt], v_all[:, ph, c, :],
                        pse[:, c, qt * 512:(qt + 1) * 512],
                        start=(c == 0), stop=(c == NKT - 1))

            def emit_norm(prev):
                ph, pse, ppvs = prev
                php, phb = ph // 2, (ph % 2) * 64
                for qt in range(NQT):
                    sums = small.tile([1, 512], F32, tag="sums")
                    nc.vector.tensor_copy(out=sums, in_=ppvs[qt][HD:HD + 1, :])
                    rec = small.tile([1, 512], F32, tag="rec")
                    nc.vector.reciprocal_approx_fast(out=rec, in_=sums)
                    rec_b = small.tile([64, 512], F32, tag="recb")
                    nc.gpsimd.partition_broadcast(rec_b, rec)
                    nc.vector.tensor_mul(
                        outn[phb:phb + 64, php, qt * 512:(qt + 1) * 512],
                        ppvs[qt][0:HD, :], rec_b)

            # software pipeline: head h's score matmuls interleave with head
            # h-1's PV matmuls so the PE never drains while ACT runs exp.
            emit_kq(0)
            emit_kq(1)
            prev = None
            for p in range(NP):
                for h01 in range(2):
                    h = 2 * p + h01
                    hb = h01 * 64
                    se = attn.tile([128, NKT, SQ], BF16, tag="se", name=f"se{h}")
                    pvs = [pspv.tile([HD + 1, 512], F32, tag="pv",
                                     name=f"pv{h}_{qt}") for qt in range(NQT)]
                    for kt in range(NKT):
                        sps = psmm.tile([128, 2, 512], F32, tag="mm",
                                        name=f"sps{h}_{kt}")
                        for qt in range(NQT):
                            nc.tensor.matmul(
                                sps[:, qt, :],
                                kT[p][hb:hb + 64, kt * 128:(kt + 1) * 128],
                                qT[p][hb:hb + 64, qt * 512:(qt + 1) * 512],
                                start=True, stop=True)
                        if prev is not None:
                            emit_pv_chunk(prev, kt)
                        nc.scalar.activation(
                            out=se[:, kt, :], in_=sps, func=AF.Exp,
                            bias=mb_t[:, kt:kt + 1])
                    if prev is not None:
                        emit_norm(prev)
                    prev = (h, se, pvs)
                    if h01 == 1 and p + 2 < NP:
                        emit_kq(p + 2)
            for c in range(NKT):
                emit_pv_chunk(prev, c)
            emit_norm(prev)

            # ---- output projection + residual + LayerNorm ----
            for st2 in range(NST // 2):
                yps = psmm.tile([128, 2, 512], F32, tag="mm", name=f"yps{st2}")
                for j in range(2):
                    st = 2 * st2 + j
                    for p in range(NP):
                        nc.tensor.matmul(
                            yps[:, j, :],
                            outn[:, p, st * 128:(st + 1) * 128],
                            wo_t[:, p, :],
                            start=(p == 0), stop=(p == NP - 1))
                for j in range(2):
                    st = 2 * st2 + j
                    xq_t = small.tile([128, D], F32, tag="xq")
                    nc.sync.dma_start(out=xq_t, in_=xq[st * 128:(st + 1) * 128, :])
                    z = small.tile([128, D], F32, tag="z")
                    nc.vector.tensor_add(z, yps[:, j, :], xq_t)
                    stats = small.tile([128, 6], F32, tag="stats")
                    nc.vector.bn_stats(out=stats, in_=z)
                    mv = small.tile([128, 2], F32, tag="mv")
                    nc.vector.bn_aggr(out=mv, in_=stats)
                    std = small.tile([128, 1], F32, tag="std")
                    nc.scalar.activation(out=std, in_=mv[:, 1:2], func=AF.Sqrt,
                                         bias=eps_t[:, 0:1])
                    rstd = small.tile([128, 1], F32, tag="rstd")
                    nc.vector.reciprocal(out=rstd, in_=std)
                    zn = small.tile([128, D], F32, tag="zn")
                    nc.gpsimd.tensor_scalar(
                        out=zn, in0=z, scalar1=mv[:, 0:1], scalar2=rstd,
                        op0=ALU.subtract, op1=ALU.mult)
                    if ln_trivial:
                        zo = zn
                    else:
                        zg = small.tile([128, D], F32, tag="z")
                        nc.gpsimd.tensor_mul(zg, zn, g_t)
                        zo = small.tile([128, D], F32, tag="zn")
                        nc.gpsimd.tensor_add(zo, zg, b_t)
                    nc.sync.dma_start(out=out[st * 128:(st + 1) * 128, :], in_=zo)

    nc.compile()
    _CACHE[key] = nc
    return nc


def _prep_in_maps(x, mask, wq, bq, wk, bk, wv, bv, wo, bo, ln_gamma, ln_beta,
                  temperature, ln_trivial, bv_trivial, S_KV):
    f32 = np.float32
    bf16 = ml_dtypes.bfloat16
    x = np.asarray(x, f32)
    mask = np.asarray(mask).astype(bool)
    wqT = np.ascontiguousarray(np.asarray(wq, f32).T).astype(bf16)
    wkT = np.ascontiguousarray(np.asarray(wk, f32).T).astype(bf16)
    wvT = np.ascontiguousarray(np.asarray(wv, f32).T).astype(bf16)
    woT = np.ascontiguousarray(np.asarray(wo, f32).T).astype(bf16)
    bq = np.asarray(bq, f32); bk = np.asarray(bk, f32)
    bv = np.asarray(bv, f32); bo = np.asarray(bo, f32)
    bqk = np.ascontiguousarray(
        np.concatenate([bq.reshape(4, 128).T, bk.reshape(4, 128).T], axis=1)
    ).astype(f32)
    temp_b = np.full((128, 1), np.asarray(temperature, f32).reshape(-1)[0], f32)

    in_maps = []
    for m in range(NCORES):
        b, half = m // 2, m % 2
        q0 = half * SQ
        xb = x[b]
        idx = np.where(~mask[b])[0]
        nkv = len(idx)
        assert nkv <= S_KV, f"unmasked keys {nkv} > S_KV={S_KV}"
        xk = np.zeros((S_KV, D), f32)
        xk[:nkv] = xb[idx]
        mbias = np.full(S_KV, -30000.0, f32)
        mbias[:nkv] = 0.0
        NKT = S_KV // 128
        im = {
            "xTk": np.ascontiguousarray(xk.T).astype(bf16),
            "xTq": np.ascontiguousarray(xb[q0:q0 + SQ].T).astype(bf16),
            "xq": np.ascontiguousarray(xb[q0:q0 + SQ] + bo[None, :]),
            "wqT": wqT, "wkT": wkT, "wvT": wvT, "woT": woT,
            "bqk": bqk,
            "maskb": np.ascontiguousarray(mbias.reshape(NKT, 128).T),
            "temp_b": temp_b,
        }
        if not bv_trivial:
            im["bv_row"] = bv.reshape(1, D).astype(bf16)
        if not ln_trivial:
            im["gamma"] = np.asarray(ln_gamma, f32).reshape(1, D)
            im["beta"] = np.asarray(ln_beta, f32).reshape(1, D)
        in_maps.append(im)
    return in_maps


def kernel(**inputs) -> np.ndarray:
    global LAST_RESULT
    ln_trivial = bool(np.all(np.asarray(inputs["ln_gamma"]) == 1.0)
                      and np.all(np.asarray(inputs["ln_beta"]) == 0.0))
    bv_trivial = bool(np.all(np.asarray(inputs["bv"]) == 0.0))
    maskarr = np.asarray(inputs["mask"]).astype(bool)
    max_unmasked = int((~maskarr).sum(axis=1).max())
    S_KV = max(256, -(-(max_unmasked + 64) // 128) * 128)
    nc = _build(ln_trivial, bv_trivial, S_KV)
    in_maps = _prep_in_maps(**inputs, ln_trivial=ln_trivial, bv_trivial=bv_trivial,
                            S_KV=S_KV)
    res = run_bass_kernel_spmd(nc, in_maps, core_ids=list(range(NCORES)),
                               trace=bool(os.environ.get("BASS_TRACE")))
    LAST_RESULT = res
    y = np.empty((B, S, D), np.float32)
    for m in range(NCORES):
        b, half = m // 2, m % 2
        y[b, half * SQ:(half + 1) * SQ] = res.results[m]["out"]
    return y


# revision 16
# speedup vs baseline: 1.2755x; 1.2755x over previous
"""Multi-head attention block (QKV proj + masked softmax + out proj + residual LN)
on 8 Trainium2 NeuronCores.

Sharding: 8 shards = (batch b, query-half); B=4, S=2048. Each core owns one
batch's full K/V and half its queries; no collectives, host concatenates.

Key compaction: masked keys contribute exactly 0 to the softmax numerator and
denominator, and key order inside the sums is irrelevant — so the host gathers
only the unmasked keys (<=1046 of 2048 here) and pads to S_KV=1280. Pad slots
get a -30000 exp bias -> exp underflows to exactly 0. Cuts score/exp/PV work
to 10/16 of full, mathematically exact.

Per-core strategy (all matmuls bf16 inputs, fp32 PSUM accumulation):
  - xT staged on host; projections contract d on partitions.
  - kT/qT per head-PAIR [128, S] (head h -> partitions (h%2)*64..);
    temperature and the k-bias are folded into the kT store.
  - scores transposed [k, q]: pad mask folded into exp bias, row sums via a
    ones-column in V. PSUM tiles 2 banks wide; one exp covers [128, 1024]
    (both q-tiles of a k-tile share the bias), halving ACT overhead.
  - PV contracts k on partitions; normalization = reciprocal_approx_fast of
    the sums row + gpsimd partition-broadcast + DVE multiply.
  - y = attn_out @ wo.T via K=128 head-pair contractions, then residual
    (+bo folded into x host-side, kept fp32) and LayerNorm (bn_stats/aggr).
  - Build-time specialization on the actual inputs: gamma==1/beta==0 and
    bv==0 drop their (otherwise dead) ops.
"""

import os
import numpy as np
import ml_dtypes

import concourse.bass as bass
import concourse.bacc as bacc
import concourse.tile as tile
import concourse.mybir as mybir
from concourse.bass_utils import run_bass_kernel_spmd

F32 = mybir.dt.float32
BF16 = mybir.dt.bfloat16
AF = mybir.ActivationFunctionType
ALU = mybir.AluOpType

B, S, D = 4, 2048, 512
H, HD = 8, 64
NCORES = 8
SQ = S // 2          # queries per core
NP = 4               # head pairs
NQT = SQ // 512      # 2 q-tiles of 512
NST = SQ // 128      # 8 output s-tiles

_CACHE = {}
LAST_RESULT = None


def _build(ln_trivial, bv_trivial, S_KV):
    NKT = S_KV // 128
    key = ("nc", ln_trivial, bv_trivial, S_KV)
    if key in _CACHE:
        return _CACHE[key]

    nc = bacc.Bacc("TRN2", target_bir_lowering=False, debug=False, num_devices=NCORES)

    xTk = nc.dram_tensor("xTk", [D, S_KV], BF16, kind="ExternalInput")
    xTq = nc.dram_tensor("xTq", [D, SQ], BF16, kind="ExternalInput")
    xq = nc.dram_tensor("xq", [SQ, D], F32, kind="ExternalInput")
    wqT = nc.dram_tensor("wqT", [D, D], BF16, kind="ExternalInput")
    wkT = nc.dram_tensor("wkT", [D, D], BF16, kind="ExternalInput")
    wvT = nc.dram_tensor("wvT", [D, D], BF16, kind="ExternalInput")
    woT = nc.dram_tensor("woT", [D, D], BF16, kind="ExternalInput")
    bqk = nc.dram_tensor("bqk", [128, 8], F32, kind="ExternalInput")
    if not bv_trivial:
        bv_row = nc.dram_tensor("bv_row", [1, D], BF16, kind="ExternalInput")
    maskb = nc.dram_tensor("maskb", [128, NKT], F32, kind="ExternalInput")
    temp_b = nc.dram_tensor("temp_b", [128, 1], F32, kind="ExternalInput")
    if not ln_trivial:
        gamma = nc.dram_tensor("gamma", [1, D], F32, kind="ExternalInput")
        beta = nc.dram_tensor("beta", [1, D], F32, kind="ExternalInput")
    out = nc.dram_tensor("out", [SQ, D], F32, kind="ExternalOutput")

    def dram_bcast(t, p=128):
        a = t.ap()
        return bass.AP(tensor=a.tensor, offset=a.offset, ap=[[0, p]] + list(a.ap)[1:])

    with tile.TileContext(nc) as tc, nc.allow_low_precision(reason="bf16 matmuls"):
        with tc.tile_pool(name="consts", bufs=1) as consts, \
             tc.tile_pool(name="kqv", bufs=1) as kqv, \
             tc.tile_pool(name="proj", bufs=1) as proj, \
             tc.tile_pool(name="attn", bufs=3) as attn, \
             tc.tile_pool(name="psmm", bufs=3, space="PSUM") as psmm, \
             tc.tile_pool(name="pspv", bufs=2, space="PSUM") as pspv, \
             tc.tile_pool(name="small", bufs=2) as small:

            # ---- constants (small, fast DMAs first) ----
            bqk_t = consts.tile([128, 8], F32, tag="bqk")
            nc.sync.dma_start(out=bqk_t, in_=bqk[:, :])
            mb_t = consts.tile([128, NKT], F32, tag="mb")
            nc.sync.dma_start(out=mb_t, in_=maskb[:, :])
            tp_t = consts.tile([128, 1], F32, tag="tp")
            nc.sync.dma_start(out=tp_t, in_=temp_b[:, :])
            if not bv_trivial:
                bv_t = consts.tile([1, D], BF16, tag="bv")
                nc.sync.dma_start(out=bv_t, in_=bv_row[:, :])
            if not ln_trivial:
                g_t = consts.tile([128, D], F32, tag="g")
                nc.sync.dma_start(out=g_t, in_=dram_bcast(gamma))
                b_t = consts.tile([128, D], F32, tag="b")
                nc.sync.dma_start(out=b_t, in_=dram_bcast(beta))
            eps_t = consts.tile([128, 1], F32, tag="eps")
            nc.vector.memset(eps_t, 1e-6)
            ones_f = consts.tile([128, 128], F32, tag="onesf")
            nc.vector.memset(ones_f, 1.0)
            ones_b = consts.tile([1, 128], BF16, tag="onesb")
            nc.vector.tensor_copy(out=ones_b, in_=ones_f[0:1, :])

            # ---- persistent activations ----
            kT = [kqv.tile([128, S_KV], BF16, tag=f"kT{p}", name=f"kT{p}")
                  for p in range(NP)]
            qT = [kqv.tile([128, SQ], BF16, tag=f"qT{p}", name=f"qT{p}")
                  for p in range(NP)]
            v_all = kqv.tile([128, H, NKT, HD + 1], BF16, tag="vall")
            outn = kqv.tile([128, NP, SQ], BF16, tag="outn")

            # ---- input staging: per-chunk DMAs so compute starts early ----
            wv_t = proj.tile([128, 4, D], BF16, tag="wv")
            xtk = proj.tile([128, 4, S_KV], BF16, tag="xtk")
            wk_t = proj.tile([128, 4, D], BF16, tag="wk")
            wq_t = proj.tile([128, 4, D], BF16, tag="wq")
            xtq = proj.tile([128, 4, SQ], BF16, tag="xtq")
            wo_t = consts.tile([128, 4, D], BF16, tag="wo")
            for c in range(4):
                nc.sync.dma_start(out=wv_t[:, c, :], in_=wvT[c * 128:(c + 1) * 128, :])
            for c in range(4):
                nc.sync.dma_start(out=xtk[:, c, :], in_=xTk[c * 128:(c + 1) * 128, :])
            for c in range(4):
                nc.sync.dma_start(out=wk_t[:, c, :], in_=wkT[c * 128:(c + 1) * 128, :])
            for c in range(4):
                nc.sync.dma_start(out=wq_t[:, c, :], in_=wqT[c * 128:(c + 1) * 128, :])
            for c in range(4):
                nc.sync.dma_start(out=xtq[:, c, :], in_=xTq[c * 128:(c + 1) * 128, :])
            for c in range(4):
                nc.sync.dma_start(out=wo_t[:, c, :], in_=woT[c * 128:(c + 1) * 128, :])

            # ---- V projection: all heads at once (+ ones-row bias matmul) ----
            nc.vector.tensor_copy(out=v_all[:, :, :, HD:HD + 1],
                                  in_=ones_f[:, 0:H * NKT])
            for t2 in range((NKT + 1) // 2):
                ts = [t for t in (2 * t2, 2 * t2 + 1) if t < NKT]
                ps = psmm.tile([128, 2, 512], F32, tag="mm")
                for j, t in enumerate(ts):
                    for c in range(4):
                        nc.tensor.matmul(
                            ps[:, j, :], xtk[:, c, t * 128:(t + 1) * 128],
                            wv_t[:, c, :], start=(c == 0),
                            stop=(c == 3 and bv_trivial))
                    if not bv_trivial:
                        nc.tensor.matmul(ps[:, j, :], ones_b[0:1, :], bv_t,
                                         start=False, stop=True)
                for h in range(H):
                    nc.vector.tensor_copy(
                        out=v_all[:, h, ts[0]:ts[0] + len(ts), 0:HD],
                        in_=ps[:, 0:len(ts), h * HD:(h + 1) * HD])

            def emit_kq(p):
                # kT store folds +bk and *temperature (exact when temp=2^-k)
                kv_groups = []
                off0 = 0
                while off0 < S_KV:
                    if S_KV - off0 >= 1024:
                        kv_groups.append((off0, (512, 512))); off0 += 1024
                    else:
                        kv_groups.append((off0, (S_KV - off0,))); off0 += S_KV - off0
                for g0, widths in kv_groups:
                    ps = psmm.tile([128, 2, 512], F32, tag="mm", name=f"psk{p}{g0}")
                    off = g0
                    for j, w in enumerate(widths):
                        for c in range(4):
                            nc.tensor.matmul(
                                ps[:, j, 0:w], wk_t[:, c, p * 128:(p + 1) * 128],
                                xtk[:, c, off:off + w],
                                start=(c == 0), stop=(c == 3))
                        off += w
                    tot = sum(widths)
                    src = ps if len(widths) == 2 else ps[:, 0, 0:tot]
                    nc.vector.tensor_scalar(
                        out=kT[p][:, g0:g0 + tot], in0=src,
                        scalar1=bqk_t[:, 4 + p:5 + p], scalar2=tp_t[:, 0:1],
                        op0=ALU.add, op1=ALU.mult)
                ps = psmm.tile([128, 2, 512], F32, tag="mm", name=f"psq{p}")
                for j in range(2):
                    for c in range(4):
                        nc.tensor.matmul(
                            ps[:, j, :], wq_t[:, c, p * 128:(p + 1) * 128],
                            xtq[:, c, j * 512:(j + 1) * 512],
                            start=(c == 0), stop=(c == 3))
                nc.vector.tensor_scalar_add(
                    out=qT[p][:, :], in0=ps, scalar1=bqk_t[:, p:p + 1])

            def emit_pv_chunk(prev, c):
                ph, pse, ppvs = prev
                for qt in range(NQT):
                    nc.tensor.matmul(
                        ppvs[qt], v_all[:, ph, c, :],
                        pse[:, c, qt * 512:(qt + 1) * 512],
                        start=(c == 0), stop=(c == NKT - 1))

            def emit_norm(prev):
                ph, pse, ppvs = prev
                php, phb = ph // 2, (ph % 2) * 64
                for qt in range(NQT):
                    sums = small.tile([1, 512], F32, tag="sums")
                    nc.vector.tensor_copy(out=sums, in_=ppvs[qt][HD:HD + 1, :])
                    rec = small.tile([1, 512], F32, tag="rec")
                    nc.vector.reciprocal_approx_fast(out=rec, in_=sums)
                    rec_b = small.tile([64, 512], F32, tag="recb")
                    nc.gpsimd.partition_broadcast(rec_b, rec)
                    nc.vector.tensor_mul(
                        outn[phb:phb + 64, php, qt * 512:(qt + 1) * 512],
                        ppvs[qt][0:HD, :], rec_b)

            # software pipeline: head h's score matmuls interleave with head
            # h-1's PV matmuls so the PE never drains while ACT runs exp.
            emit_kq(0)
            emit_kq(1)
            prev = None
            for p in range(NP):
                for h01 in range(2):
                    h = 2 * p + h01
                    hb = h01 * 64
                    se = attn.tile([128, NKT, SQ], BF16, tag="se", name=f"se{h}")
                    pvs = [pspv.tile([HD + 1, 512], F32, tag="pv",
                                     name=f"pv{h}_{qt}") for qt in range(NQT)]
                    for kt in range(NKT):
                        sps = psmm.tile([128, 2, 512], F32, tag="mm",
                                        name=f"sps{h}_{kt}")
                        for qt in range(NQT):
                            nc.tensor.matmul(
                                sps[:, qt, :],
                                kT[p][hb:hb + 64, kt * 128:(kt + 1) * 128],
                                qT[p][hb:hb + 64, qt * 512:(qt + 1) * 512],
                                start=True, stop=True)
                        if prev is not None:
                            emit_pv_chunk(prev, kt)
                        nc.scalar.activation(
                            out=se[:, kt, :], in_=sps, func=AF.Exp,
                            bias=mb_t[:, kt:kt + 1])
                    if prev is not None:
                        emit_norm(prev)
                    prev = (h, se, pvs)
                    if h01 == 1 and p + 2 < NP:
                        emit_kq(p + 2)
            for c in range(NKT):
                emit_pv_chunk(prev, c)
            emit_norm(prev)

            # ---- output projection + residual + LayerNorm ----
            for st2 in range(NST // 2):
                yps = psmm.tile([128, 2, 512], F32, tag="mm", name=f"yps{st2}")
                for j in range(2):
                    st = 2 * st2 + j
                    for p in range(NP):
                        nc.tensor.matmul(
                            yps[:, j, :],
                            outn[:, p, st * 128:(st + 1) * 128],
                            wo_t[:, p, :],
                            start=(p == 0), stop=(p == NP - 1))
                for j in range(2):
                    st = 2 * st2 + j
                    xq_t = small.tile([128, D], F32, tag="xq")
                    nc.sync.dma_start(out=xq_t, in_=xq[st * 128:(st + 1) * 128, :])
                    z = small.tile([128, D], F32, tag="z")
                    nc.vector.tensor_add(z, yps[:, j, :], xq_t)
                    stats = small.tile([128, 6], F32, tag="stats")
                    nc.vector.bn_stats(out=stats, in_=z)
                    mv = small.tile([128, 2], F32, tag="mv")
                    nc.vector.bn_aggr(out=mv, in_=stats)
                    std = small.tile([128, 1], F32, tag="std")
                    nc.scalar.activation(out=std, in_=mv[:, 1:2], func=AF.Sqrt,
                                         bias=eps_t[:, 0:1])
                    rstd = small.tile([128, 1], F32, tag="rstd")
                    nc.vector.reciprocal(out=rstd, in_=std)
                    zn = small.tile([128, D], F32, tag="zn")
                    nc.vector.tensor_scalar(
                        out=zn, in0=z, scalar1=mv[:, 0:1], scalar2=rstd,
                        op0=ALU.subtract, op1=ALU.mult)
                    if ln_trivial:
                        zo = zn
                    else:
                        zg = small.tile([128, D], F32, tag="z")
                        nc.gpsimd.tensor_mul(zg, zn, g_t)
                        zo = small.tile([128, D], F32, tag="zn")
                        nc.gpsimd.tensor_add(zo, zg, b_t)
                    nc.sync.dma_start(out=out[st * 128:(st + 1) * 128, :], in_=zo)

    nc.compile()
    _CACHE[key] = nc
    return nc


def _prep_in_maps(x, mask, wq, bq, wk, bk, wv, bv, wo, bo, ln_gamma, ln_beta,
                  temperature, ln_trivial, bv_trivial, S_KV):
    f32 = np.float32
    bf16 = ml_dtypes.bfloat16
    x = np.asarray(x, f32)
    mask = np.asarray(mask).astype(bool)
    wqT = np.ascontiguousarray(np.asarray(wq, f32).T).astype(bf16)
    wkT = np.ascontiguousarray(np.asarray(wk, f32).T).astype(bf16)
    wvT = np.ascontiguousarray(np.asarray(wv, f32).T).astype(bf16)
    woT = np.ascontiguousarray(np.asarray(wo, f32).T).astype(bf16)
    bq = np.asarray(bq, f32); bk = np.asarray(bk, f32)
    bv = np.asarray(bv, f32); bo = np.asarray(bo, f32)
    bqk = np.ascontiguousarray(
        np.concatenate([bq.reshape(4, 128).T, bk.reshape(4, 128).T], axis=1)
    ).astype(f32)
    temp_b = np.full((128, 1), np.asarray(temperature, f32).reshape(-1)[0], f32)

    in_maps = []
    for m in range(NCORES):
        b, half = m // 2, m % 2
        q0 = half * SQ
        xb = x[b]
        idx = np.where(~mask[b])[0]
        nkv = len(idx)
        assert nkv <= S_KV, f"unmasked keys {nkv} > S_KV={S_KV}"
        xk = np.zeros((S_KV, D), f32)
        xk[:nkv] = xb[idx]
        mbias = np.full(S_KV, -30000.0, f32)
        mbias[:nkv] = 0.0
        NKT = S_KV // 128
        im = {
            "xTk": np.ascontiguousarray(xk.T).astype(bf16),
            "xTq": np.ascontiguousarray(xb[q0:q0 + SQ].T).astype(bf16),
            "xq": np.ascontiguousarray(xb[q0:q0 + SQ] + bo[None, :]),
            "wqT": wqT, "wkT": wkT, "wvT": wvT, "woT": woT,
            "bqk": bqk,
            "maskb": np.ascontiguousarray(mbias.reshape(NKT, 128).T),
            "temp_b": temp_b,
        }
        if not bv_trivial:
            im["bv_row"] = bv.reshape(1, D).astype(bf16)
        if not ln_trivial:
            im["gamma"] = np.asarray(ln_gamma, f32).reshape(1, D)
            im["beta"] = np.asarray(ln_beta, f32).reshape(1, D)
        in_maps.append(im)
    return in_maps


def kernel(**inputs) -> np.ndarray:
    global LAST_RESULT
    ln_trivial = bool(np.all(np.asarray(inputs["ln_gamma"]) == 1.0)
                      and np.all(np.asarray(inputs["ln_beta"]) == 0.0))
    bv_trivial = bool(np.all(np.asarray(inputs["bv"]) == 0.0))
    maskarr = np.asarray(inputs["mask"]).astype(bool)
    max_unmasked = int((~maskarr).sum(axis=1).max())
    S_KV = max(256, -(-(max_unmasked + 64) // 128) * 128)
    nc = _build(ln_trivial, bv_trivial, S_KV)
    in_maps = _prep_in_maps(**inputs, ln_trivial=ln_trivial, bv_trivial=bv_trivial,
                            S_KV=S_KV)
    res = run_bass_kernel_spmd(nc, in_maps, core_ids=list(range(NCORES)),
                               trace=bool(os.environ.get("BASS_TRACE")))
    LAST_RESULT = res
    y = np.empty((B, S, D), np.float32)
    for m in range(NCORES):
        b, half = m // 2, m % 2
        y[b, half * SQ:(half + 1) * SQ] = res.results[m]["out"]
    return y


# revision 17
# speedup vs baseline: 1.3543x; 1.0617x over previous
"""Multi-head attention block (QKV proj + masked softmax + out proj + residual LN)
on 8 Trainium2 NeuronCores.

Sharding: 8 shards = (batch b, query-half); B=4, S=2048. Each core owns one
batch's full K/V and half its queries; no collectives, host concatenates.

Key compaction: masked keys contribute exactly 0 to the softmax numerator and
denominator, and key order inside the sums is irrelevant — so the host gathers
only the unmasked keys (<=1046 of 2048 here) and pads to S_KV=1280. Pad slots
get a -30000 exp bias -> exp underflows to exactly 0. Cuts score/exp/PV work
to 10/16 of full, mathematically exact.

Per-core strategy (all matmuls bf16 inputs, fp32 PSUM accumulation):
  - xT staged on host; projections contract d on partitions.
  - kT/qT per head-PAIR [128, S] (head h -> partitions (h%2)*64..);
    temperature and the k-bias are folded into the kT store.
  - scores transposed [k, q]: pad mask folded into exp bias, row sums via a
    ones-column in V. PSUM tiles 2 banks wide; one exp covers [128, 1024]
    (both q-tiles of a k-tile share the bias), halving ACT overhead.
  - PV contracts k on partitions; normalization = reciprocal_approx_fast of
    the sums row + gpsimd partition-broadcast + DVE multiply.
  - y = attn_out @ wo.T via K=128 head-pair contractions, then residual
    (+bo folded into x host-side, kept fp32) and LayerNorm (bn_stats/aggr).
  - Build-time specialization on the actual inputs: gamma==1/beta==0 and
    bv==0 drop their (otherwise dead) ops.
"""

import os
import numpy as np
import ml_dtypes

import concourse.bass as bass
import concourse.bacc as bacc
import concourse.tile as tile
import concourse.mybir as mybir
from concourse.bass_utils import run_bass_kernel_spmd

F32 = mybir.dt.float32
BF16 = mybir.dt.bfloat16
AF = mybir.ActivationFunctionType
ALU = mybir.AluOpType

B, S, D = 4, 2048, 512
H, HD = 8, 64
NCORES = 8
SQ = S // 2          # queries per core
NP = 4               # head pairs
NQT = SQ // 512      # 2 q-tiles of 512
NST = SQ // 128      # 8 output s-tiles

_CACHE = {}
LAST_RESULT = None


def _build(ln_trivial, bv_trivial, S_KV):
    NKT = S_KV // 128
    key = ("nc", ln_trivial, bv_trivial, S_KV)
    if key in _CACHE:
        return _CACHE[key]

    nc = bacc.Bacc("TRN2", target_bir_lowering=False, debug=False, num_devices=NCORES)

    xTk = nc.dram_tensor("xTk", [D, S_KV], BF16, kind="ExternalInput")
    xTq = nc.dram_tensor("xTq", [D, SQ], BF16, kind="ExternalInput")
    xq = nc.dram_tensor("xq", [SQ, D], F32, kind="ExternalInput")
    wqT = nc.dram_tensor("wqT", [D, D], BF16, kind="ExternalInput")
    wkT = nc.dram_tensor("wkT", [D, D], BF16, kind="ExternalInput")
    wvT = nc.dram_tensor("wvT", [D, D], BF16, kind="ExternalInput")
    woT = nc.dram_tensor("woT", [D, D], BF16, kind="ExternalInput")
    bqk = nc.dram_tensor("bqk", [128, 8], F32, kind="ExternalInput")
    if not bv_trivial:
        bv_row = nc.dram_tensor("bv_row", [1, D], BF16, kind="ExternalInput")
    maskb = nc.dram_tensor("maskb", [128, NKT], F32, kind="ExternalInput")
    temp_b = nc.dram_tensor("temp_b", [128, 1], F32, kind="ExternalInput")
    eye = nc.dram_tensor("eye", [128, 128], F32, kind="ExternalInput")
    if not ln_trivial:
        gamma = nc.dram_tensor("gamma", [1, D], F32, kind="ExternalInput")
        beta = nc.dram_tensor("beta", [1, D], F32, kind="ExternalInput")
    out = nc.dram_tensor("out", [SQ, D], F32, kind="ExternalOutput")

    def dram_bcast(t, p=128):
        a = t.ap()
        return bass.AP(tensor=a.tensor, offset=a.offset, ap=[[0, p]] + list(a.ap)[1:])

    with tile.TileContext(nc) as tc, nc.allow_low_precision(reason="bf16 matmuls"):
        with tc.tile_pool(name="consts", bufs=1) as consts, \
             tc.tile_pool(name="kqv", bufs=1) as kqv, \
             tc.tile_pool(name="proj", bufs=1) as proj, \
             tc.tile_pool(name="attn", bufs=3) as attn, \
             tc.tile_pool(name="psmm", bufs=3, space="PSUM") as psmm, \
             tc.tile_pool(name="pspv", bufs=2, space="PSUM") as pspv, \
             tc.tile_pool(name="small", bufs=2) as small:

            # ---- constants (small, fast DMAs first) ----
            bqk_t = consts.tile([128, 8], F32, tag="bqk")
            nc.sync.dma_start(out=bqk_t, in_=bqk[:, :])
            mb_t = consts.tile([128, NKT], F32, tag="mb")
            nc.sync.dma_start(out=mb_t, in_=maskb[:, :])
            tp_t = consts.tile([128, 1], F32, tag="tp")
            nc.sync.dma_start(out=tp_t, in_=temp_b[:, :])
            if not bv_trivial:
                bv_t = consts.tile([1, D], BF16, tag="bv")
                nc.sync.dma_start(out=bv_t, in_=bv_row[:, :])
            if not ln_trivial:
                g_t = consts.tile([128, D], F32, tag="g")
                nc.sync.dma_start(out=g_t, in_=dram_bcast(gamma))
                b_t = consts.tile([128, D], F32, tag="b")
                nc.sync.dma_start(out=b_t, in_=dram_bcast(beta))
            eye_t = consts.tile([128, 128], F32, tag="eye")
            nc.sync.dma_start(out=eye_t, in_=eye[:, :])
            eps_t = consts.tile([128, 1], F32, tag="eps")
            nc.vector.memset(eps_t, 1e-6)
            ones_f = consts.tile([128, 128], F32, tag="onesf")
            nc.vector.memset(ones_f, 1.0)
            ones_b = consts.tile([1, 128], BF16, tag="onesb")
            nc.vector.tensor_copy(out=ones_b, in_=ones_f[0:1, :])

            # ---- persistent activations ----
            kT = [kqv.tile([128, S_KV], BF16, tag=f"kT{p}", name=f"kT{p}")
                  for p in range(NP)]
            qT = [kqv.tile([128, SQ], BF16, tag=f"qT{p}", name=f"qT{p}")
                  for p in range(NP)]
            v_all = kqv.tile([128, H, NKT, HD + 1], BF16, tag="vall")
            outn = kqv.tile([128, NP, SQ], BF16, tag="outn")

            # ---- input staging: per-chunk DMAs so compute starts early ----
            wv_t = proj.tile([128, 4, D], BF16, tag="wv")
            xtk = proj.tile([128, 4, S_KV], BF16, tag="xtk")
            wk_t = proj.tile([128, 4, D], BF16, tag="wk")
            wq_t = proj.tile([128, 4, D], BF16, tag="wq")
            xtq = proj.tile([128, 4, SQ], BF16, tag="xtq")
            wo_t = consts.tile([128, 4, D], BF16, tag="wo")
            for c in range(4):
                nc.sync.dma_start(out=xtk[:, c, :], in_=xTk[c * 128:(c + 1) * 128, :])
            for c in range(4):
                nc.sync.dma_start(out=wv_t[:, c, :], in_=wvT[c * 128:(c + 1) * 128, :])
            for c in range(4):
                nc.sync.dma_start(out=wk_t[:, c, :], in_=wkT[c * 128:(c + 1) * 128, :])
            for c in range(4):
                nc.sync.dma_start(out=wq_t[:, c, :], in_=wqT[c * 128:(c + 1) * 128, :])
            for c in range(4):
                nc.sync.dma_start(out=xtq[:, c, :], in_=xTq[c * 128:(c + 1) * 128, :])
            for c in range(4):
                nc.sync.dma_start(out=wo_t[:, c, :], in_=woT[c * 128:(c + 1) * 128, :])

            # ---- V projection: all heads at once (+ ones-row bias matmul) ----
            nc.vector.tensor_copy(out=v_all[:, :, :, HD:HD + 1],
                                  in_=ones_f[:, 0:H * NKT])
            for t2 in range((NKT + 1) // 2):
                ts = [t for t in (2 * t2, 2 * t2 + 1) if t < NKT]
                ps = psmm.tile([128, 2, 512], F32, tag="mm")
                for j, t in enumerate(ts):
                    for c in range(4):
                        nc.tensor.matmul(
                            ps[:, j, :], xtk[:, c, t * 128:(t + 1) * 128],
                            wv_t[:, c, :], start=(c == 0),
                            stop=(c == 3 and bv_trivial))
                    if not bv_trivial:
                        nc.tensor.matmul(ps[:, j, :], ones_b[0:1, :], bv_t,
                                         start=False, stop=True)
                for h in range(H):
                    nc.vector.tensor_copy(
                        out=v_all[:, h, ts[0]:ts[0] + len(ts), 0:HD],
                        in_=ps[:, 0:len(ts), h * HD:(h + 1) * HD])

            def emit_kq(p):
                # kT store folds +bk and *temperature (exact when temp=2^-k)
                kv_groups = []
                off0 = 0
                while off0 < S_KV:
                    if S_KV - off0 >= 1024:
                        kv_groups.append((off0, (512, 512))); off0 += 1024
                    else:
                        kv_groups.append((off0, (S_KV - off0,))); off0 += S_KV - off0
                for g0, widths in kv_groups:
                    ps = psmm.tile([128, 2, 512], F32, tag="mm", name=f"psk{p}{g0}")
                    off = g0
                    for j, w in enumerate(widths):
                        for c in range(4):
                            nc.tensor.matmul(
                                ps[:, j, 0:w], wk_t[:, c, p * 128:(p + 1) * 128],
                                xtk[:, c, off:off + w],
                                start=(c == 0), stop=(c == 3))
                        off += w
                    tot = sum(widths)
                    src = ps if len(widths) == 2 else ps[:, 0, 0:tot]
                    nc.vector.tensor_scalar(
                        out=kT[p][:, g0:g0 + tot], in0=src,
                        scalar1=bqk_t[:, 4 + p:5 + p], scalar2=tp_t[:, 0:1],
                        op0=ALU.add, op1=ALU.mult)
                ps = psmm.tile([128, 2, 512], F32, tag="mm", name=f"psq{p}")
                for j in range(2):
                    for c in range(4):
                        nc.tensor.matmul(
                            ps[:, j, :], wq_t[:, c, p * 128:(p + 1) * 128],
                            xtq[:, c, j * 512:(j + 1) * 512],
                            start=(c == 0), stop=(c == 3))
                nc.vector.tensor_scalar_add(
                    out=qT[p][:, :], in0=ps, scalar1=bqk_t[:, p:p + 1])

            def emit_pv_chunk(prev, c):
                ph, pse, ppvs = prev
                for qt in range(NQT):
                    nc.tensor.matmul(
                        ppvs[qt], v_all[:, ph, c, :],
                        pse[:, c, qt * 512:(qt + 1) * 512],
                        start=(c == 0), stop=(c == NKT - 1))

            def emit_norm(prev):
                ph, pse, ppvs = prev
                php, phb = ph // 2, (ph % 2) * 64
                for qt in range(NQT):
                    sums = small.tile([1, 512], F32, tag="sums")
                    nc.vector.tensor_copy(out=sums, in_=ppvs[qt][HD:HD + 1, :])
                    rec = small.tile([1, 512], F32, tag="rec")
                    nc.vector.reciprocal_approx_fast(out=rec, in_=sums)
                    rec_b = small.tile([64, 512], F32, tag="recb")
                    nc.gpsimd.partition_broadcast(rec_b, rec)
                    nc.vector.tensor_mul(
                        outn[phb:phb + 64, php, qt * 512:(qt + 1) * 512],
                        ppvs[qt][0:HD, :], rec_b)

            # software pipeline: head h's score matmuls interleave with head
            # h-1's PV matmuls so the PE never drains while ACT runs exp.
            emit_kq(0)
            emit_kq(1)
            prev = None
            for p in range(NP):
                for h01 in range(2):
                    h = 2 * p + h01
                    hb = h01 * 64
                    se = attn.tile([128, NKT, SQ], BF16, tag="se", name=f"se{h}")
                    pvs = [pspv.tile([HD + 1, 512], F32, tag="pv",
                                     name=f"pv{h}_{qt}") for qt in range(NQT)]
                    for kt in range(NKT):
                        sps = psmm.tile([128, 2, 512], F32, tag="mm",
                                        name=f"sps{h}_{kt}")
                        for qt in range(NQT):
                            nc.tensor.matmul(
                                sps[:, qt, :],
                                kT[p][hb:hb + 64, kt * 128:(kt + 1) * 128],
                                qT[p][hb:hb + 64, qt * 512:(qt + 1) * 512],
                                start=True, stop=True)
                        if prev is not None:
                            emit_pv_chunk(prev, kt)
                        nc.scalar.activation(
                            out=se[:, kt, :], in_=sps, func=AF.Exp,
                            bias=mb_t[:, kt:kt + 1])
                    if prev is not None:
                        emit_norm(prev)
                    prev = (h, se, pvs)
                    if h01 == 1 and p + 2 < NP:
                        emit_kq(p + 2)
            for c in range(NKT):
                emit_pv_chunk(prev, c)
            emit_norm(prev)

            # ---- output projection + residual + LayerNorm ----
            xq_tiles = []
            for st in range(NST):
                xq_t = small.tile([128, D], F32, tag=f"xq{st}", name=f"xq{st}")
                nc.sync.dma_start(out=xq_t, in_=xq[st * 128:(st + 1) * 128, :])
                xq_tiles.append(xq_t)
            for st2 in range(NST // 2):
                yps = psmm.tile([128, 2, 512], F32, tag="mm", name=f"yps{st2}")
                for j in range(2):
                    st = 2 * st2 + j
                    for p in range(NP):
                        nc.tensor.matmul(
                            yps[:, j, :],
                            outn[:, p, st * 128:(st + 1) * 128],
                            wo_t[:, p, :],
                            start=(p == 0), stop=False)
                    nc.tensor.matmul(yps[:, j, :], eye_t, xq_tiles[st],
                                     start=False, stop=True)
                for j in range(2):
                    st = 2 * st2 + j
                    z = yps[:, j, :]
                    stats = small.tile([128, 6], F32, tag="stats")
                    nc.vector.bn_stats(out=stats, in_=z)
                    mv = small.tile([128, 2], F32, tag="mv")
                    nc.vector.bn_aggr(out=mv, in_=stats)
                    std = small.tile([128, 1], F32, tag="std")
                    nc.scalar.activation(out=std, in_=mv[:, 1:2], func=AF.Sqrt,
                                         bias=eps_t[:, 0:1])
                    rstd = small.tile([128, 1], F32, tag="rstd")
                    nc.vector.reciprocal(out=rstd, in_=std)
                    zn = small.tile([128, D], F32, tag="zn")
                    nc.vector.tensor_scalar(
                        out=zn, in0=z, scalar1=mv[:, 0:1], scalar2=rstd,
                        op0=ALU.subtract, op1=ALU.mult)
                    if ln_trivial:
                        zo = zn
                    else:
                        zg = small.tile([128, D], F32, tag="z")
                        nc.gpsimd.tensor_mul(zg, zn, g_t)
                        zo = small.tile([128, D], F32, tag="zn")
                        nc.gpsimd.tensor_add(zo, zg, b_t)
                    nc.sync.dma_start(out=out[st * 128:(st + 1) * 128, :], in_=zo)

    nc.compile()
    _CACHE[key] = nc
    return nc


def _prep_in_maps(x, mask, wq, bq, wk, bk, wv, bv, wo, bo, ln_gamma, ln_beta,
                  temperature, ln_trivial, bv_trivial, S_KV):
    f32 = np.float32
    bf16 = ml_dtypes.bfloat16
    x = np.asarray(x, f32)
    mask = np.asarray(mask).astype(bool)
    wqT = np.ascontiguousarray(np.asarray(wq, f32).T).astype(bf16)
    wkT = np.ascontiguousarray(np.asarray(wk, f32).T).astype(bf16)
    wvT = np.ascontiguousarray(np.asarray(wv, f32).T).astype(bf16)
    woT = np.ascontiguousarray(np.asarray(wo, f32).T).astype(bf16)
    bq = np.asarray(bq, f32); bk = np.asarray(bk, f32)
    bv = np.asarray(bv, f32); bo = np.asarray(bo, f32)
    bqk = np.ascontiguousarray(
        np.concatenate([bq.reshape(4, 128).T, bk.reshape(4, 128).T], axis=1)
    ).astype(f32)
    temp_b = np.full((128, 1), np.asarray(temperature, f32).reshape(-1)[0], f32)

    in_maps = []
    for m in range(NCORES):
        b, half = m // 2, m % 2
        q0 = half * SQ
        xb = x[b]
        idx = np.where(~mask[b])[0]
        nkv = len(idx)
        assert nkv <= S_KV, f"unmasked keys {nkv} > S_KV={S_KV}"
        xk = np.zeros((S_KV, D), f32)
        xk[:nkv] = xb[idx]
        mbias = np.full(S_KV, -30000.0, f32)
        mbias[:nkv] = 0.0
        NKT = S_KV // 128
        im = {
            "eye": np.eye(128, dtype=f32),
            "xTk": np.ascontiguousarray(xk.T).astype(bf16),
            "xTq": np.ascontiguousarray(xb[q0:q0 + SQ].T).astype(bf16),
            "xq": np.ascontiguousarray(xb[q0:q0 + SQ] + bo[None, :]),
            "wqT": wqT, "wkT": wkT, "wvT": wvT, "woT": woT,
            "bqk": bqk,
            "maskb": np.ascontiguousarray(mbias.reshape(NKT, 128).T),
            "temp_b": temp_b,
        }
        if not bv_trivial:
            im["bv_row"] = bv.reshape(1, D).astype(bf16)
        if not ln_trivial:
            im["gamma"] = np.asarray(ln_gamma, f32).reshape(1, D)
            im["beta"] = np.asarray(ln_beta, f32).reshape(1, D)
        in_maps.append(im)
    return in_maps


def kernel(**inputs) -> np.ndarray:
    global LAST_RESULT
    ln_trivial = bool(np.all(np.asarray(inputs["ln_gamma"]) == 1.0)
                      and np.all(np.asarray(inputs["ln_beta"]) == 0.0))
    bv_trivial = bool(np.all(np.asarray(inputs["bv"]) == 0.0))
    maskarr = np.asarray(inputs["mask"]).astype(bool)
    max_unmasked = int((~maskarr).sum(axis=1).max())
    S_KV = max(256, -(-(max_unmasked + 64) // 128) * 128)
    nc = _build(ln_trivial, bv_trivial, S_KV)
    in_maps = _prep_in_maps(**inputs, ln_trivial=ln_trivial, bv_trivial=bv_trivial,
                            S_KV=S_KV)
    res = run_bass_kernel_spmd(nc, in_maps, core_ids=list(range(NCORES)),
                               trace=bool(os.environ.get("BASS_TRACE")))
    LAST_RESULT = res
    y = np.empty((B, S, D), np.float32)
    for m in range(NCORES):
        b, half = m // 2, m % 2
        y[b, half * SQ:(half + 1) * SQ] = res.results[m]["out"]
    return y
